# revision 4
# baseline (speedup 1.0000x reference)
"""Multi-head attention (RoPE, causal) Trainium2 Bass kernel, v2.

Sharding: 8 cores = DP(2 batches) x TP(4 head-quads of 4 heads each).

v2 structural changes vs v1:
- No separate "ones" sum-matmuls: each vsb tile packs per head
  [64 v-cols | 64 ones-cols]; the M=128 ctx matmul then yields rows
  0:64 = unnormalized ctx and rows 64:128 = 64 replicas of sumexp
  (matmul cost depends only on N, so the replicas are free).  This
  removes ~34us of Tensor-engine time.
- Normalization per (pair, block): DVE reciprocal on the replica rows
  -> rec [64,512], then two [64,512] muls -> csb bf16.
- Scores psum per sk-tile [128, 1024] = (h0 | h1), double-buffered so
  exp never stalls the PE; one exp activation per tile.
- e tiles allocated as [128, 2048] pairs (2 sk-tiles) to cut act count.
- Diagonal-tile trimming: a causal diagonal sk-tile's leading fully-
  masked score columns (128*c for the c-th diagonal tile of a block)
  are skipped in the scores matmul, exp, mask multiply and ctx matmul
  (~10us PE + ~10us Act + ~7us DVE).
- Output written bf16 (host sums partials in f32 and adds bo).
- DMA order: xq+wq first so the first projection matmul starts ~1.5us
  in; constants follow.  v-evictions and some output evictions moved to
  the otherwise-idle Pool engine.

kernel(**inputs) takes FULL unsharded numpy inputs, returns FULL
[B, S, D] float32 output.
"""

import sys

if "/opt/trn_rl_repo" not in sys.path:
    sys.path.insert(0, "/opt/trn_rl_repo")

import numpy as np
import ml_dtypes

import concourse.bass as bass
import concourse.bacc as bacc
import concourse.mybir as mybir
import concourse.tile as tile
from concourse.bass_utils import run_bass_kernel_spmd

BF16 = mybir.dt.bfloat16
F32 = mybir.dt.float32
NPBF16 = ml_dtypes.bfloat16

B, S, D, H, DK = 2, 2048, 1024, 16, 64
NCORES = 8
TP = 4            # head-quads per batch
HPC = H // TP     # heads per core = 4
OC = HPC * DK     # q/k/v projection output dims per core = 256
NPAIR = HPC // 2  # head pairs per core = 2
NB = S // 512     # sq blocks of width 512
NT = S // 128     # sk tiles of width 128
ND = D // 128     # contraction d-tiles

last_exec_time_ns = None
_cache = {}


def _rope_tables():
    """COS/SSIN tables [128, S]: per head pair, rows [0:32]=cos/-sin(freq j),
    [32:64]=cos/+sin, repeated for the second head."""
    a = np.arange(0, DK, 2, dtype=np.float32)
    inv_freq = (10000.0 ** (-2.0 * a / DK)).astype(np.float32)
    pos = np.arange(S, dtype=np.float32)
    ang = pos[:, None] * inv_freq[None, :]
    cos = np.cos(ang).T.astype(np.float32)
    sin = np.sin(ang).T.astype(np.float32)
    cos128 = np.concatenate([cos, cos, cos, cos], axis=0)
    sin128 = np.concatenate([sin, -sin, sin, -sin], axis=0)
    return cos128, sin128


def _analyze_mask(mask):
    """Classify [sk_tile 128] x [sq_block 512] blocks of the attention mask.

    blocks[b] = list of (t, mid); mask_tiles[i] is a [128, 512] bf16 0/1
    multiplier laid out [sk partition, sq free]."""
    m = np.asarray(mask).reshape(S, S)
    blocks = []
    tiles = []
    keys = {}
    for b in range(NB):
        cur = []
        for t in range(NT):
            sub = (m[512 * b:512 * b + 512, 128 * t:128 * t + 128] != 0)
            if not sub.any():
                continue
            if sub.all():
                cur.append((t, None, 0))
                continue
            tl = np.ascontiguousarray(sub.T).astype(NPBF16)
            k = tl.tobytes()
            if k not in keys:
                keys[k] = len(tiles)
                tiles.append(tl)
            colsum = (tl != 0).any(axis=0)
            lead = int(np.argmax(colsum)) if colsum.any() else 0
            cur.append((t, keys[k], lead))
        blocks.append(cur)
    return blocks, tiles


def _build_nc(blocks, n_masks, qk_bias=False, v_bias=False, loop_n=None,
              markers=None, loop_scope="all", dma_only=False):
    nc = bacc.Bacc(None)

    def mark(label):
        if markers is not None:
            n = int(nc.get_next_instruction_name().split("-")[1])
            markers.append((n, label))

    xq = nc.declare_dram_parameter("xqT", [D, S], BF16, isOutput=False)
    xk = nc.declare_dram_parameter("xkT", [D, S], BF16, isOutput=False)
    xv = nc.declare_dram_parameter("xvT", [D, S], BF16, isOutput=False)
    wq = nc.declare_dram_parameter("wqT", [D, OC], BF16, isOutput=False)
    wk = nc.declare_dram_parameter("wkT", [D, OC], BF16, isOutput=False)
    wv = nc.declare_dram_parameter("wvT", [D, OC], BF16, isOutput=False)
    wo = nc.declare_dram_parameter("woT", [OC, D], BF16, isOutput=False)
    cosd = nc.declare_dram_parameter("cos", [128, S], BF16, isOutput=False)
    ssind = nc.declare_dram_parameter("ssin", [128, S], BF16, isOutput=False)
    bqd = nc.declare_dram_parameter("bq", [128, NPAIR], F32, isOutput=False)
    bkd = nc.declare_dram_parameter("bk", [128, NPAIR], F32, isOutput=False)
    bvd = nc.declare_dram_parameter("bv", [128, OC], F32, isOutput=False)
    nm = max(n_masks, 1)
    maskd = nc.declare_dram_parameter("masks", [nm, 128, 512], BF16,
                                      isOutput=False)
    outp = nc.declare_dram_parameter("out", [S, D], BF16, isOutput=True)

    with tile.TileContext(nc) as tc:
        from contextlib import ExitStack
        with ExitStack() as ctx:
            ep = ctx.enter_context
            const = ep(tc.tile_pool(name="const", bufs=1))
            xt_p = ep(tc.tile_pool(name="xt", bufs=12))
            w_p = ep(tc.tile_pool(name="w", bufs=10))
            rope_p = ep(tc.tile_pool(name="rope", bufs=6))
            hat_p = ep(tc.tile_pool(name="hat", bufs=4))
            vsb_p = ep(tc.tile_pool(name="vsb", bufs=17))
            e_p = ep(tc.tile_pool(name="e", bufs=9))
            ctx_p = ep(tc.tile_pool(name="ctxsb", bufs=6))
            rec_p = ep(tc.tile_pool(name="rec", bufs=2))
            out_p = ep(tc.tile_pool(name="outsb", bufs=4))
            sc_ps = ep(tc.tile_pool(name="sc", bufs=2, space="PSUM"))
            cx_ps = ep(tc.tile_pool(name="cx", bufs=4, space="PSUM"))
            if loop_n is not None and loop_scope == "all":
                ep(tc.For_i(0, loop_n, 1))

            mark("dma")
            # ---- input DMAs: x/w tiles first so matmuls start early ----
            xt = {}
            wt = {}
            for name, xd, wd in (("q", xq, wq), ("k", xk, wk), ("v", xv, wv)):
                for dt in range(ND):
                    x_t = xt_p.tile([128, S], BF16, tag="xt")
                    nc.sync.dma_start(out=x_t, in_=xd[128 * dt:128 * dt + 128, :])
                    xt[(name, dt)] = x_t
                    w_t = w_p.tile([128, OC], BF16, tag="w")
                    nc.gpsimd.dma_start(out=w_t, in_=wd[128 * dt:128 * dt + 128, :])
                    wt[(name, dt)] = w_t
                if name == "q":
                    cos_sb = const.tile([128, S], BF16)
                    ssin_sb = const.tile([128, S], BF16)
                    nc.gpsimd.dma_start(out=cos_sb, in_=cosd[:, :])
                    nc.gpsimd.dma_start(out=ssin_sb, in_=ssind[:, :])

            wo_sb = []
            for p in range(NPAIR):
                w_t = const.tile([128, D], BF16, tag=f"wo{p}")
                nc.gpsimd.dma_start(out=w_t, in_=wo[128 * p:128 * p + 128, :])
                wo_sb.append(w_t)
            mask_sb = []
            for i in range(nm):
                m_t = const.tile([128, 512], BF16, tag=f"mask{i}")
                nc.gpsimd.dma_start(out=m_t, in_=maskd[i])
                mask_sb.append(m_t)
            if qk_bias:
                bq_sb = const.tile([128, NPAIR], F32)
                bk_sb = const.tile([128, NPAIR], F32)
                nc.gpsimd.dma_start(out=bq_sb, in_=bqd[:, :])
                nc.gpsimd.dma_start(out=bk_sb, in_=bkd[:, :])
            if v_bias:
                bv_sb = const.tile([128, OC], F32)
                nc.gpsimd.dma_start(out=bv_sb, in_=bvd[:, :])

            if loop_n is not None and loop_scope == "compute":
                ep(tc.For_i(0, loop_n, 1))

            mark("qkproj")
            # ---- q/k projections -> raw [o_p, s_f] + RoPE -> hats ----
            hats = {}
            ropes = {}

            def emit_rope(name, p, c0, c1):
                raw, t1, t2 = ropes[(name, p)]
                cs = slice(c0, c1)
                nc.vector.tensor_mul(t1[:, cs], raw[:, cs], cos_sb[:, cs])
                nc.vector.tensor_mul(t2[0:32, cs], raw[32:64, cs],
                                     ssin_sb[32:64, cs])
                nc.vector.tensor_mul(t2[32:64, cs], raw[0:32, cs],
                                     ssin_sb[0:32, cs])
                nc.vector.tensor_mul(t2[64:96, cs], raw[96:128, cs],
                                     ssin_sb[96:128, cs])
                nc.vector.tensor_mul(t2[96:128, cs], raw[64:96, cs],
                                     ssin_sb[64:96, cs])
                nc.vector.tensor_add(t1[:, cs], t1[:, cs], t2[:, cs])

            for name in ("q", "k"):
                bias_sb = None
                if qk_bias:
                    bias_sb = bq_sb if name == "q" else bk_sb
                # 8 concurrent psum streams (2 pairs x 4 sb chunks), matmuls
                # issued dt-major so they keep pace with the x-tile DMAs.
                raws = {}
                scs = {}
                cxs = {}
                for p in range(NPAIR):
                    raw = rope_p.tile([128, S], BF16, tag="raw")
                    raws[p] = raw
                    pssc = sc_ps.tile([128, 1024], F32, tag="sc")
                    scs[p] = pssc
                    pcx0 = cx_ps.tile([128, 512], F32, tag="cx")
                    pcx1 = cx_ps.tile([128, 512], F32, tag="cx")
                    cxs[p] = (pcx0, pcx1)
                for dt in range(ND):
                    for p in range(NPAIR):
                        lhs = wt[(name, dt)][:, 128 * p:128 * p + 128]
                        st_ = (dt == 0)
                        sp_ = (dt == ND - 1)
                        nc.tensor.matmul(
                            scs[p][:, 0:512], lhsT=lhs,
                            rhs=xt[(name, dt)][:, 0:512],
                            start=st_, stop=sp_, skip_group_check=True)
                        nc.tensor.matmul(
                            scs[p][:, 512:1024], lhsT=lhs,
                            rhs=xt[(name, dt)][:, 512:1024],
                            start=st_, stop=sp_, skip_group_check=True)
                        nc.tensor.matmul(
                            cxs[p][0], lhsT=lhs,
                            rhs=xt[(name, dt)][:, 1024:1536],
                            start=st_, stop=sp_, skip_group_check=True)
                        nc.tensor.matmul(
                            cxs[p][1], lhsT=lhs,
                            rhs=xt[(name, dt)][:, 1536:2048],
                            start=st_, stop=sp_, skip_group_check=True)
                for p in range(NPAIR):
                    raw = raws[p]
                    if qk_bias:
                        idn = mybir.ActivationFunctionType.Identity
                        nc.scalar.activation(raw[:, 0:1024], scs[p], idn,
                                             bias=bias_sb[:, p:p + 1])
                        nc.scalar.activation(raw[:, 1024:1536], cxs[p][0], idn,
                                             bias=bias_sb[:, p:p + 1])
                        nc.scalar.activation(raw[:, 1536:2048], cxs[p][1], idn,
                                             bias=bias_sb[:, p:p + 1])
                    else:
                        nc.scalar.copy(raw[:, 0:1024], scs[p])
                        nc.scalar.copy(raw[:, 1024:1536], cxs[p][0])
                        nc.scalar.copy(raw[:, 1536:2048], cxs[p][1])
                    # RoPE: hat[e] = raw[e]*cos - raw[o]*sin, hat[o] = ...
                    # t2 written with partition-shifted outputs; sign baked
                    # into the ssin table rows.  For k, emitted in column
                    # halves with both pairs' first halves first, so blk1
                    # (sk tiles 0..7) unblocks before the second halves run.
                    t1 = hat_p.tile([128, S], BF16, tag="hat")
                    t2 = rope_p.tile([128, S], BF16, tag="t2")
                    hats[(name, p)] = t1
                    ropes[(name, p)] = (raw, t1, t2)

            vsb = []

            # ---- attention + output projection ----
            # e tiles in pairs of sk tiles: [128, (t0h0 t0h1 t1h0 t1h1)]
            def emit_scores_grp(b, p, grp, first_grp=False):
                qh = hats[("q", p)]
                kh = hats[("k", p)]
                epair = e_p.tile([128, 2048], BF16, tag="e")
                for c, (t, mid, lead) in enumerate(grp):
                    if first_grp and c == 0:
                        lead = 0
                    ps = sc_ps.tile([128, 1024], F32, tag="sc")
                    nc.tensor.matmul(
                        ps[:, lead:512],
                        lhsT=kh[0:64, 128 * t:128 * t + 128],
                        rhs=qh[0:64, 512 * b + lead:512 * b + 512],
                        start=True, stop=True, tile_position=(0, 0))
                    nc.tensor.matmul(
                        ps[:, 512 + lead:1024],
                        lhsT=kh[64:128, 128 * t:128 * t + 128],
                        rhs=qh[64:128, 512 * b + lead:512 * b + 512],
                        start=True, stop=True, tile_position=(64, 0))
                    if lead:
                        ps3 = ps.rearrange("p (h x) -> p h x", h=2)
                        e3 = epair[:, 1024 * c:1024 * c + 1024].rearrange(
                            "p (h x) -> p h x", h=2)
                        nc.scalar.activation(
                            e3[:, :, lead:512], ps3[:, :, lead:512],
                            mybir.ActivationFunctionType.Exp)
                    else:
                        nc.scalar.activation(
                            epair[:, 1024 * c:1024 * c + 1024], ps,
                            mybir.ActivationFunctionType.Exp)
                    if mid is not None:
                        for h in range(2):
                            o0 = 1024 * c + 512 * h
                            nc.vector.tensor_mul(
                                epair[:, o0 + lead:o0 + 512],
                                epair[:, o0 + lead:o0 + 512],
                                mask_sb[mid][:, lead:512])
                return epair

            def emit_ctx_grp(b, p, grp, epair, cps, gi, n):
                for c, (t, mid, lead) in enumerate(grp):
                    first = (gi == 0)
                    last = (gi == n - 1)
                    if first:
                        lead = 0  # psum group start must cover the full range
                    for h in range(2):
                        nc.tensor.matmul(
                            cps[h][:, lead:512],
                            lhsT=vsb[t][:, 128 * h + 256 * p:
                                        128 * h + 256 * p + 128],
                            rhs=epair[:, 1024 * c + 512 * h + lead:
                                      1024 * c + 512 * h + 512],
                            start=first, stop=last,
                            tile_position=(0, 0),
                            skip_group_check=True)
                    gi += 1
                return gi

            def emit_norm(cps):
                # rows 64:128 of cps[h] hold sumexp replicas
                csb = ctx_p.tile([128, 512], BF16, tag="ctxsb")
                for h in range(2):
                    rec = rec_p.tile([64, 512], F32, tag="rec")
                    nc.vector.reciprocal(rec, cps[h][64:128, :])
                    nc.vector.tensor_mul(csb[64 * h:64 * h + 64, :],
                                         cps[h][0:64, :], rec)
                return csb

            oev = [0]

            def emit_wo(b, csbs):
                for j in range(4):
                    for oh in range(2):
                        ps = cx_ps.tile([128, 512], F32, tag="cx")
                        for p in range(NPAIR):
                            nc.tensor.matmul(
                                ps,
                                lhsT=csbs[p][:, 128 * j:128 * j + 128],
                                rhs=wo_sb[p][:, 512 * oh:512 * oh + 512],
                                start=(p == 0), stop=(p == NPAIR - 1))
                        o_t = out_p.tile([128, 512], BF16, tag="outsb")
                        if oev[0] % 2 == 0:
                            nc.vector.tensor_copy(o_t, ps)
                        else:
                            nc.scalar.copy(o_t, ps)
                        dma_eng = nc.sync if oev[0] % 2 == 0 else nc.gpsimd
                        dma_eng.dma_start(
                            out=outp[512 * b + 128 * j:512 * b + 128 * j + 128,
                                     512 * oh:512 * oh + 512],
                            in_=o_t)
                        oev[0] += 1

            def emit_block(b):
                mark(f"blk{b}")
                act = blocks[b]
                n = len(act)
                csbs = []
                for p in range(NPAIR):
                    cps0 = cx_ps.tile([128, 512], F32, tag="cx")
                    cps1 = cx_ps.tile([128, 512], F32, tag="cx")
                    cps = [cps0, cps1]
                    gi = 0
                    for g0 in range(0, n, 2):
                        grp = act[g0:g0 + 2]
                        epair = emit_scores_grp(b, p, grp, first_grp=(g0 == 0))
                        gi = emit_ctx_grp(b, p, grp, epair, cps, gi, n)
                    csbs.append(emit_norm(cps))
                emit_wo(b, csbs)

            for p in range(NPAIR):
                emit_rope("q", p, 0, S)
            emit_rope("k", 0, 0, S)
            emit_rope("k", 1, 0, S)

            mark("vproj")
            # ---- v projection -> vsb tiles [128sk, 4*(64v|64ones)] ----
            # waves of 4 concurrent psum streams, matmuls dt-major so a
            # late-arriving xv tile doesn't head-of-line-block ready work
            # 4 wave psums (bank-aligned [128,512] cx tiles, one group
            # each); waves rotate as Act-engine evictions free them
            for w in range(NT // 4):
                vps0 = cx_ps.tile([128, 512], F32, tag="cx")
                vps1 = cx_ps.tile([128, 512], F32, tag="cx")
                vps2 = cx_ps.tile([128, 512], F32, tag="cx")
                vps3 = cx_ps.tile([128, 512], F32, tag="cx")
                pss = [vps0, vps1, vps2, vps3]
                for dt in range(ND):
                    for i in range(4):
                        st = 4 * w + i
                        nc.tensor.matmul(
                            pss[i][:, 0:OC],
                            lhsT=xt[("v", dt)][:, 128 * st:128 * st + 128],
                            rhs=wt[("v", dt)][:, :],
                            start=(dt == 0), stop=(dt == ND - 1),
                            skip_group_check=True)
                for i in range(4):
                    st = 4 * w + i
                    v_t = vsb_p.tile([128, 512], BF16, tag="vsb")
                    v3 = v_t.rearrange("p (h x) -> p h x", h=HPC)
                    s3 = pss[i][:, 0:OC].rearrange("p (h x) -> p h x", h=HPC)
                    if v_bias:
                        b3 = bv_sb.rearrange("p (h x) -> p h x", h=HPC)
                        nc.vector.tensor_add(v3[:, :, 0:DK], s3, b3)
                    elif st % 2 == 0:
                        nc.scalar.copy(v3[:, :, 0:DK], s3)
                    else:
                        nc.vector.tensor_copy(v3[:, :, 0:DK], s3)
                    nc.gpsimd.memset(v3[:, :, DK:128], 1.0)
                    vsb.append(v_t)

            emit_block(1)
            emit_block(2)
            emit_block(3)
            emit_block(0)
    nc.finalize()
    return nc


def _prep_core_inputs(inputs, blocks, mask_tiles):
    """Build the 8 per-core input maps (host-side sharding)."""
    q = np.asarray(inputs["q"], np.float32)
    k = np.asarray(inputs["k"], np.float32)
    v = np.asarray(inputs["v"], np.float32)
    Wq = np.asarray(inputs["Wq"], np.float32)
    Wk = np.asarray(inputs["Wk"], np.float32)
    Wv = np.asarray(inputs["Wv"], np.float32)
    Wo = np.asarray(inputs["Wo"], np.float32)
    bq = np.asarray(inputs["bq"], np.float32)
    bk = np.asarray(inputs["bk"], np.float32)
    bv = np.asarray(inputs["bv"], np.float32)

    cos128, ssin128 = _rope_tables()
    cos_b = cos128.astype(NPBF16)
    ssin_b = ssin128.astype(NPBF16)
    nm = max(len(mask_tiles), 1)
    masks_t = np.zeros((nm, 128, 512), NPBF16)
    for i, t in enumerate(mask_tiles):
        masks_t[i] = t

    # de-interleave permutation within each head: evens then odds
    perm64 = np.concatenate([np.arange(0, DK, 2), np.arange(1, DK, 2)])

    xT = {}
    for bb in range(B):
        xT[("q", bb)] = np.ascontiguousarray(q[bb].T).astype(NPBF16)
        xT[("k", bb)] = np.ascontiguousarray(k[bb].T).astype(NPBF16)
        xT[("v", bb)] = np.ascontiguousarray(v[bb].T).astype(NPBF16)

    scale = np.float32(1.0 / np.sqrt(DK))
    in_maps = []
    for c in range(NCORES):
        bb, hq = divmod(c, TP)
        rows = []
        for h in range(HPC):
            base = OC * hq + DK * h
            rows.extend((base + perm64).tolist())
        rows = np.array(rows)
        cols = np.arange(OC * hq, OC * hq + OC)

        wqT = np.ascontiguousarray(Wq[rows, :].T).astype(NPBF16)
        wkT = np.ascontiguousarray((Wk[rows, :] * scale).T).astype(NPBF16)
        wvT = np.ascontiguousarray(Wv[cols, :].T).astype(NPBF16)
        woT = np.ascontiguousarray(Wo[:, cols].T).astype(NPBF16)
        bq_t = np.ascontiguousarray(bq[rows].reshape(NPAIR, 128).T).astype(np.float32)
        bk_t = np.ascontiguousarray((bk[rows] * scale).reshape(NPAIR, 128).T).astype(np.float32)
        bv_t = np.broadcast_to(bv[cols], (128, OC)).astype(np.float32)

        in_maps.append({
            "xqT": xT[("q", bb)], "xkT": xT[("k", bb)], "xvT": xT[("v", bb)],
            "wqT": wqT, "wkT": wkT, "wvT": wvT, "woT": woT,
            "cos": cos_b, "ssin": ssin_b,
            "bq": bq_t, "bk": bk_t, "bv": bv_t,
            "masks": masks_t,
        })
    return in_maps


def kernel(**inputs):
    global last_exec_time_ns
    import os

    mask = np.asarray(inputs["mask"])
    blocks, mask_tiles = _analyze_mask(mask)
    qk_bias = bool(np.any(np.asarray(inputs["bq"])) or np.any(np.asarray(inputs["bk"])))
    v_bias = bool(np.any(np.asarray(inputs["bv"])))
    key = (tuple(tuple(bl) for bl in blocks), len(mask_tiles), qk_bias, v_bias)
    if key not in _cache:
        _cache[key] = _build_nc(blocks, len(mask_tiles), qk_bias, v_bias)
    nc = _cache[key]

    in_maps = _prep_core_inputs(inputs, blocks, mask_tiles)
    trace = bool(os.environ.get("KERNEL_TRACE"))
    import time
    last_err = None
    for attempt in range(3):
        try:
            res = run_bass_kernel_spmd(nc, in_maps, list(range(NCORES)),
                                       trace=trace)
            break
        except Exception as e:  # transient NRT device-unrecoverable wedges
            last_err = e
            time.sleep(10.0)
    else:
        raise last_err
    last_exec_time_ns = res.exec_time_ns

    bo = np.asarray(inputs["bo"], np.float32)
    out = np.zeros((B, S, D), np.float32)
    for c in range(NCORES):
        bb = c // TP
        out[bb] += res.results[c]["out"].astype(np.float32)
    out += bo[None, None, :]
    return out


# revision 5
# speedup vs baseline: 1.0329x; 1.0329x over previous
"""Multi-head attention (RoPE, causal) Trainium2 Bass kernel, v2.

Sharding: 8 cores = DP(2 batches) x TP(4 head-quads of 4 heads each).

v2 structural changes vs v1:
- No separate "ones" sum-matmuls: each vsb tile packs per head
  [64 v-cols | 64 ones-cols]; the M=128 ctx matmul then yields rows
  0:64 = unnormalized ctx and rows 64:128 = 64 replicas of sumexp
  (matmul cost depends only on N, so the replicas are free).  This
  removes ~34us of Tensor-engine time.
- Normalization per (pair, block): DVE reciprocal on the replica rows
  -> rec [64,512], then two [64,512] muls -> csb bf16.
- Scores psum per sk-tile [128, 1024] = (h0 | h1), double-buffered so
  exp never stalls the PE; one exp activation per tile.
- e tiles allocated as [128, 2048] pairs (2 sk-tiles) to cut act count.
- Output written bf16 (host sums partials in f32 and adds bo).
- DMA order: xq+wq first so the first projection matmul starts ~1.5us
  in; constants follow.  v-evictions and some output evictions moved to
  the otherwise-idle Pool engine.

kernel(**inputs) takes FULL unsharded numpy inputs, returns FULL
[B, S, D] float32 output.
"""

import sys

if "/opt/trn_rl_repo" not in sys.path:
    sys.path.insert(0, "/opt/trn_rl_repo")

import numpy as np
import ml_dtypes

import concourse.bass as bass
import concourse.bacc as bacc
import concourse.mybir as mybir
import concourse.tile as tile
from concourse.bass_utils import run_bass_kernel_spmd

BF16 = mybir.dt.bfloat16
F32 = mybir.dt.float32
NPBF16 = ml_dtypes.bfloat16

B, S, D, H, DK = 2, 2048, 1024, 16, 64
NCORES = 8
TP = 4            # head-quads per batch
HPC = H // TP     # heads per core = 4
OC = HPC * DK     # q/k/v projection output dims per core = 256
NPAIR = HPC // 2  # head pairs per core = 2
NB = S // 512     # sq blocks of width 512
NT = S // 128     # sk tiles of width 128
ND = D // 128     # contraction d-tiles

last_exec_time_ns = None
_cache = {}


def _rope_tables():
    """COS/SSIN tables [128, S]: per head pair, rows [0:32]=cos/-sin(freq j),
    [32:64]=cos/+sin, repeated for the second head."""
    a = np.arange(0, DK, 2, dtype=np.float32)
    inv_freq = (10000.0 ** (-2.0 * a / DK)).astype(np.float32)
    pos = np.arange(S, dtype=np.float32)
    ang = pos[:, None] * inv_freq[None, :]
    cos = np.cos(ang).T.astype(np.float32)
    sin = np.sin(ang).T.astype(np.float32)
    cos128 = np.concatenate([cos, cos, cos, cos], axis=0)
    sin128 = np.concatenate([sin, -sin, sin, -sin], axis=0)
    return cos128, sin128


def _analyze_mask(mask):
    """Classify [sk_tile 128] x [sq_block 512] blocks of the attention mask.

    blocks[b] = list of (t, mid); mask_tiles[i] is a [128, 512] bf16 0/1
    multiplier laid out [sk partition, sq free]."""
    m = np.asarray(mask).reshape(S, S)
    blocks = []
    tiles = []
    keys = {}
    for b in range(NB):
        cur = []
        for t in range(NT):
            sub = (m[512 * b:512 * b + 512, 128 * t:128 * t + 128] != 0)
            if not sub.any():
                continue
            if sub.all():
                cur.append((t, None, 0))
                continue
            tl = np.ascontiguousarray(sub.T).astype(NPBF16)
            k = tl.tobytes()
            if k not in keys:
                keys[k] = len(tiles)
                tiles.append(tl)
            colsum = (tl != 0).any(axis=0)
            lead = int(np.argmax(colsum)) if colsum.any() else 0
            cur.append((t, keys[k], lead))
        blocks.append(cur)
    return blocks, tiles


def _build_nc(blocks, n_masks, qk_bias=False, v_bias=False, loop_n=None,
              markers=None, loop_scope="all", dma_only=False):
    nc = bacc.Bacc(None)

    def mark(label):
        if markers is not None:
            n = int(nc.get_next_instruction_name().split("-")[1])
            markers.append((n, label))

    xq = nc.declare_dram_parameter("xqT", [D, S], BF16, isOutput=False)
    xk = nc.declare_dram_parameter("xkT", [D, S], BF16, isOutput=False)
    xv = nc.declare_dram_parameter("xvT", [D, S], BF16, isOutput=False)
    wq = nc.declare_dram_parameter("wqT", [D, OC], BF16, isOutput=False)
    wk = nc.declare_dram_parameter("wkT", [D, OC], BF16, isOutput=False)
    wv = nc.declare_dram_parameter("wvT", [D, OC], BF16, isOutput=False)
    wo = nc.declare_dram_parameter("woT", [OC, D], BF16, isOutput=False)
    cosd = nc.declare_dram_parameter("cos", [128, S], BF16, isOutput=False)
    ssind = nc.declare_dram_parameter("ssin", [128, S], BF16, isOutput=False)
    bqd = nc.declare_dram_parameter("bq", [128, NPAIR], F32, isOutput=False)
    bkd = nc.declare_dram_parameter("bk", [128, NPAIR], F32, isOutput=False)
    bvd = nc.declare_dram_parameter("bv", [128, OC], F32, isOutput=False)
    nm = max(n_masks, 1)
    maskd = nc.declare_dram_parameter("masks", [nm, 128, 512], BF16,
                                      isOutput=False)
    outp = nc.declare_dram_parameter("out", [S, D], BF16, isOutput=True)

    with tile.TileContext(nc) as tc:
        from contextlib import ExitStack
        with ExitStack() as ctx:
            ep = ctx.enter_context
            const = ep(tc.tile_pool(name="const", bufs=1))
            xt_p = ep(tc.tile_pool(name="xt", bufs=12))
            w_p = ep(tc.tile_pool(name="w", bufs=10))
            rope_p = ep(tc.tile_pool(name="rope", bufs=6))
            hat_p = ep(tc.tile_pool(name="hat", bufs=4))
            vsb_p = ep(tc.tile_pool(name="vsb", bufs=17))
            e_p = ep(tc.tile_pool(name="e", bufs=9))
            ctx_p = ep(tc.tile_pool(name="ctxsb", bufs=6))
            rec_p = ep(tc.tile_pool(name="rec", bufs=2))
            out_p = ep(tc.tile_pool(name="outsb", bufs=4))
            sc_ps = ep(tc.tile_pool(name="sc", bufs=2, space="PSUM"))
            cx_ps = ep(tc.tile_pool(name="cx", bufs=4, space="PSUM"))
            if loop_n is not None and loop_scope == "all":
                ep(tc.For_i(0, loop_n, 1))

            mark("dma")
            # ---- input DMAs: x/w tiles first so matmuls start early ----
            xt = {}
            wt = {}
            for name, xd, wd in (("q", xq, wq), ("k", xk, wk), ("v", xv, wv)):
                for dt in range(ND):
                    x_t = xt_p.tile([128, S], BF16, tag="xt")
                    xq_eng = nc.gpsimd if name == "v" else nc.sync
                    xq_eng.dma_start(out=x_t, in_=xd[128 * dt:128 * dt + 128, :])
                    xt[(name, dt)] = x_t
                    w_t = w_p.tile([128, OC], BF16, tag="w")
                    nc.gpsimd.dma_start(out=w_t, in_=wd[128 * dt:128 * dt + 128, :])
                    wt[(name, dt)] = w_t
                if name == "q":
                    cos_sb = const.tile([128, S], BF16)
                    ssin_sb = const.tile([128, S], BF16)
                    nc.gpsimd.dma_start(out=cos_sb, in_=cosd[:, :])
                    nc.gpsimd.dma_start(out=ssin_sb, in_=ssind[:, :])

            wo_sb = []
            for p in range(NPAIR):
                w_t = const.tile([128, D], BF16, tag=f"wo{p}")
                nc.gpsimd.dma_start(out=w_t, in_=wo[128 * p:128 * p + 128, :])
                wo_sb.append(w_t)
            mask_sb = []
            for i in range(nm):
                m_t = const.tile([128, 512], BF16, tag=f"mask{i}")
                nc.gpsimd.dma_start(out=m_t, in_=maskd[i])
                mask_sb.append(m_t)
            if qk_bias:
                bq_sb = const.tile([128, NPAIR], F32)
                bk_sb = const.tile([128, NPAIR], F32)
                nc.gpsimd.dma_start(out=bq_sb, in_=bqd[:, :])
                nc.gpsimd.dma_start(out=bk_sb, in_=bkd[:, :])
            if v_bias:
                bv_sb = const.tile([128, OC], F32)
                nc.gpsimd.dma_start(out=bv_sb, in_=bvd[:, :])

            if loop_n is not None and loop_scope == "compute":
                ep(tc.For_i(0, loop_n, 1))

            mark("qkproj")
            # ---- q/k projections -> raw [o_p, s_f] + RoPE -> hats ----
            hats = {}
            ropes = {}

            def emit_rope(name, p, c0, c1):
                raw, t1, t2 = ropes[(name, p)]
                cs = slice(c0, c1)
                nc.vector.tensor_mul(t1[:, cs], raw[:, cs], cos_sb[:, cs])
                nc.vector.tensor_mul(t2[0:32, cs], raw[32:64, cs],
                                     ssin_sb[32:64, cs])
                nc.vector.tensor_mul(t2[32:64, cs], raw[0:32, cs],
                                     ssin_sb[0:32, cs])
                nc.vector.tensor_mul(t2[64:96, cs], raw[96:128, cs],
                                     ssin_sb[96:128, cs])
                nc.vector.tensor_mul(t2[96:128, cs], raw[64:96, cs],
                                     ssin_sb[64:96, cs])
                nc.vector.tensor_add(t1[:, cs], t1[:, cs], t2[:, cs])

            for name in ("q", "k"):
                bias_sb = None
                if qk_bias:
                    bias_sb = bq_sb if name == "q" else bk_sb
                # 8 concurrent psum streams (2 pairs x 4 sb chunks), matmuls
                # issued dt-major so they keep pace with the x-tile DMAs.
                raws = {}
                scs = {}
                cxs = {}
                for p in range(NPAIR):
                    raw = rope_p.tile([128, S], BF16, tag="raw")
                    raws[p] = raw
                    pssc = sc_ps.tile([128, 1024], F32, tag="sc")
                    scs[p] = pssc
                    pcx0 = cx_ps.tile([128, 512], F32, tag="cx")
                    pcx1 = cx_ps.tile([128, 512], F32, tag="cx")
                    cxs[p] = (pcx0, pcx1)
                for dt in range(ND):
                    for p in range(NPAIR):
                        lhs = wt[(name, dt)][:, 128 * p:128 * p + 128]
                        st_ = (dt == 0)
                        sp_ = (dt == ND - 1)
                        nc.tensor.matmul(
                            scs[p][:, 0:512], lhsT=lhs,
                            rhs=xt[(name, dt)][:, 0:512],
                            start=st_, stop=sp_, skip_group_check=True)
                        nc.tensor.matmul(
                            scs[p][:, 512:1024], lhsT=lhs,
                            rhs=xt[(name, dt)][:, 512:1024],
                            start=st_, stop=sp_, skip_group_check=True)
                        nc.tensor.matmul(
                            cxs[p][0], lhsT=lhs,
                            rhs=xt[(name, dt)][:, 1024:1536],
                            start=st_, stop=sp_, skip_group_check=True)
                        nc.tensor.matmul(
                            cxs[p][1], lhsT=lhs,
                            rhs=xt[(name, dt)][:, 1536:2048],
                            start=st_, stop=sp_, skip_group_check=True)
                for p in range(NPAIR):
                    raw = raws[p]
                    if qk_bias:
                        idn = mybir.ActivationFunctionType.Identity
                        nc.scalar.activation(raw[:, 0:1024], scs[p], idn,
                                             bias=bias_sb[:, p:p + 1])
                        nc.scalar.activation(raw[:, 1024:1536], cxs[p][0], idn,
                                             bias=bias_sb[:, p:p + 1])
                        nc.scalar.activation(raw[:, 1536:2048], cxs[p][1], idn,
                                             bias=bias_sb[:, p:p + 1])
                    else:
                        nc.scalar.copy(raw[:, 0:1024], scs[p])
                        nc.scalar.copy(raw[:, 1024:1536], cxs[p][0])
                        nc.scalar.copy(raw[:, 1536:2048], cxs[p][1])
                    # RoPE: hat[e] = raw[e]*cos - raw[o]*sin, hat[o] = ...
                    # t2 written with partition-shifted outputs; sign baked
                    # into the ssin table rows.  For k, emitted in column
                    # halves with both pairs' first halves first, so blk1
                    # (sk tiles 0..7) unblocks before the second halves run.
                    t1 = hat_p.tile([128, S], BF16, tag="hat")
                    t2 = rope_p.tile([128, S], BF16, tag="t2")
                    hats[(name, p)] = t1
                    ropes[(name, p)] = (raw, t1, t2)

            vsb = []

            # ---- attention + output projection ----
            # e tiles in pairs of sk tiles: [128, (t0h0 t0h1 t1h0 t1h1)]
            def emit_scores_grp(b, p, grp, first_grp=False):
                qh = hats[("q", p)]
                kh = hats[("k", p)]
                epair = e_p.tile([128, 2048], BF16, tag="e")
                for c, (t, mid, lead) in enumerate(grp):
                    if first_grp and c == 0:
                        lead = 0
                    ps = sc_ps.tile([128, 1024], F32, tag="sc")
                    nc.tensor.matmul(
                        ps[:, lead:512],
                        lhsT=kh[0:64, 128 * t:128 * t + 128],
                        rhs=qh[0:64, 512 * b + lead:512 * b + 512],
                        start=True, stop=True, tile_position=(0, 0))
                    nc.tensor.matmul(
                        ps[:, 512 + lead:1024],
                        lhsT=kh[64:128, 128 * t:128 * t + 128],
                        rhs=qh[64:128, 512 * b + lead:512 * b + 512],
                        start=True, stop=True, tile_position=(64, 0))
                    if lead:
                        ps3 = ps.rearrange("p (h x) -> p h x", h=2)
                        e3 = epair[:, 1024 * c:1024 * c + 1024].rearrange(
                            "p (h x) -> p h x", h=2)
                        nc.scalar.activation(
                            e3[:, :, lead:512], ps3[:, :, lead:512],
                            mybir.ActivationFunctionType.Exp)
                    else:
                        nc.scalar.activation(
                            epair[:, 1024 * c:1024 * c + 1024], ps,
                            mybir.ActivationFunctionType.Exp)
                    if mid is not None:
                        for h in range(2):
                            o0 = 1024 * c + 512 * h
                            nc.vector.tensor_mul(
                                epair[:, o0 + lead:o0 + 512],
                                epair[:, o0 + lead:o0 + 512],
                                mask_sb[mid][:, lead:512])
                return epair

            def emit_ctx_grp(b, p, grp, epair, cps, gi, n):
                for c, (t, mid, lead) in enumerate(grp):
                    first = (gi == 0)
                    last = (gi == n - 1)
                    if first:
                        lead = 0  # psum group start must cover the full range
                    for h in range(2):
                        nc.tensor.matmul(
                            cps[h][:, lead:512],
                            lhsT=vsb[t][:, 128 * h + 256 * p:
                                        128 * h + 256 * p + 128],
                            rhs=epair[:, 1024 * c + 512 * h + lead:
                                      1024 * c + 512 * h + 512],
                            start=first, stop=last,
                            tile_position=(0, 0),
                            skip_group_check=True)
                    gi += 1
                return gi

            def emit_norm(cps):
                # rows 64:128 of cps[h] hold sumexp replicas
                csb = ctx_p.tile([128, 512], BF16, tag="ctxsb")
                for h in range(2):
                    rec = rec_p.tile([64, 512], F32, tag="rec")
                    nc.vector.reciprocal(rec, cps[h][64:128, :])
                    nc.vector.tensor_mul(csb[64 * h:64 * h + 64, :],
                                         cps[h][0:64, :], rec)
                return csb

            oev = [0]

            def emit_wo(b, csbs):
                for j in range(4):
                    for oh in range(2):
                        ps = cx_ps.tile([128, 512], F32, tag="cx")
                        for p in range(NPAIR):
                            nc.tensor.matmul(
                                ps,
                                lhsT=csbs[p][:, 128 * j:128 * j + 128],
                                rhs=wo_sb[p][:, 512 * oh:512 * oh + 512],
                                start=(p == 0), stop=(p == NPAIR - 1))
                        o_t = out_p.tile([128, 512], BF16, tag="outsb")
                        if oev[0] % 2 == 0:
                            nc.vector.tensor_copy(o_t, ps)
                        else:
                            nc.scalar.copy(o_t, ps)
                        dma_eng = nc.sync if oev[0] % 2 == 0 else nc.gpsimd
                        dma_eng.dma_start(
                            out=outp[512 * b + 128 * j:512 * b + 128 * j + 128,
                                     512 * oh:512 * oh + 512],
                            in_=o_t)
                        oev[0] += 1

            def emit_block(b):
                mark(f"blk{b}")
                act = blocks[b]
                n = len(act)
                csbs = []
                for p in range(NPAIR):
                    cps0 = cx_ps.tile([128, 512], F32, tag="cx")
                    cps1 = cx_ps.tile([128, 512], F32, tag="cx")
                    cps = [cps0, cps1]
                    gi = 0
                    for g0 in range(0, n, 2):
                        grp = act[g0:g0 + 2]
                        epair = emit_scores_grp(b, p, grp, first_grp=(g0 == 0))
                        gi = emit_ctx_grp(b, p, grp, epair, cps, gi, n)
                    csbs.append(emit_norm(cps))
                emit_wo(b, csbs)

            for p in range(NPAIR):
                emit_rope("q", p, 0, S)
            emit_rope("k", 0, 0, S)
            emit_rope("k", 1, 0, S)

            mark("vproj")
            # ---- v projection -> vsb tiles [128sk, 4*(64v|64ones)] ----
            # waves of 4 concurrent psum streams, matmuls dt-major so a
            # late-arriving xv tile doesn't head-of-line-block ready work
            # 4 wave psums (bank-aligned [128,512] cx tiles, one group
            # each); waves rotate as Act-engine evictions free them
            for w in range(NT // 4):
                vps0 = cx_ps.tile([128, 512], F32, tag="cx")
                vps1 = cx_ps.tile([128, 512], F32, tag="cx")
                vps2 = cx_ps.tile([128, 512], F32, tag="cx")
                vps3 = cx_ps.tile([128, 512], F32, tag="cx")
                pss = [vps0, vps1, vps2, vps3]
                for dt in range(ND):
                    for i in range(4):
                        st = 4 * w + i
                        nc.tensor.matmul(
                            pss[i][:, 0:OC],
                            lhsT=xt[("v", dt)][:, 128 * st:128 * st + 128],
                            rhs=wt[("v", dt)][:, :],
                            start=(dt == 0), stop=(dt == ND - 1),
                            skip_group_check=True)
                for i in range(4):
                    st = 4 * w + i
                    v_t = vsb_p.tile([128, 512], BF16, tag="vsb")
                    v3 = v_t.rearrange("p (h x) -> p h x", h=HPC)
                    s3 = pss[i][:, 0:OC].rearrange("p (h x) -> p h x", h=HPC)
                    if v_bias:
                        b3 = bv_sb.rearrange("p (h x) -> p h x", h=HPC)
                        nc.vector.tensor_add(v3[:, :, 0:DK], s3, b3)
                    elif st % 2 == 0:
                        nc.scalar.copy(v3[:, :, 0:DK], s3)
                    else:
                        nc.vector.tensor_copy(v3[:, :, 0:DK], s3)
                    nc.gpsimd.memset(v3[:, :, DK:128], 1.0)
                    vsb.append(v_t)

            emit_block(1)
            emit_block(0)
            emit_block(2)
            emit_block(3)
    nc.finalize()
    return nc


def _prep_core_inputs(inputs, blocks, mask_tiles):
    """Build the 8 per-core input maps (host-side sharding)."""
    q = np.asarray(inputs["q"], np.float32)
    k = np.asarray(inputs["k"], np.float32)
    v = np.asarray(inputs["v"], np.float32)
    Wq = np.asarray(inputs["Wq"], np.float32)
    Wk = np.asarray(inputs["Wk"], np.float32)
    Wv = np.asarray(inputs["Wv"], np.float32)
    Wo = np.asarray(inputs["Wo"], np.float32)
    bq = np.asarray(inputs["bq"], np.float32)
    bk = np.asarray(inputs["bk"], np.float32)
    bv = np.asarray(inputs["bv"], np.float32)

    cos128, ssin128 = _rope_tables()
    cos_b = cos128.astype(NPBF16)
    ssin_b = ssin128.astype(NPBF16)
    nm = max(len(mask_tiles), 1)
    masks_t = np.zeros((nm, 128, 512), NPBF16)
    for i, t in enumerate(mask_tiles):
        masks_t[i] = t

    # de-interleave permutation within each head: evens then odds
    perm64 = np.concatenate([np.arange(0, DK, 2), np.arange(1, DK, 2)])

    xT = {}
    for bb in range(B):
        xT[("q", bb)] = np.ascontiguousarray(q[bb].T).astype(NPBF16)
        xT[("k", bb)] = np.ascontiguousarray(k[bb].T).astype(NPBF16)
        xT[("v", bb)] = np.ascontiguousarray(v[bb].T).astype(NPBF16)

    scale = np.float32(1.0 / np.sqrt(DK))
    in_maps = []
    for c in range(NCORES):
        bb, hq = divmod(c, TP)
        rows = []
        for h in range(HPC):
            base = OC * hq + DK * h
            rows.extend((base + perm64).tolist())
        rows = np.array(rows)
        cols = np.arange(OC * hq, OC * hq + OC)

        wqT = np.ascontiguousarray(Wq[rows, :].T).astype(NPBF16)
        wkT = np.ascontiguousarray((Wk[rows, :] * scale).T).astype(NPBF16)
        wvT = np.ascontiguousarray(Wv[cols, :].T).astype(NPBF16)
        woT = np.ascontiguousarray(Wo[:, cols].T).astype(NPBF16)
        bq_t = np.ascontiguousarray(bq[rows].reshape(NPAIR, 128).T).astype(np.float32)
        bk_t = np.ascontiguousarray((bk[rows] * scale).reshape(NPAIR, 128).T).astype(np.float32)
        bv_t = np.broadcast_to(bv[cols], (128, OC)).astype(np.float32)

        in_maps.append({
            "xqT": xT[("q", bb)], "xkT": xT[("k", bb)], "xvT": xT[("v", bb)],
            "wqT": wqT, "wkT": wkT, "wvT": wvT, "woT": woT,
            "cos": cos_b, "ssin": ssin_b,
            "bq": bq_t, "bk": bk_t, "bv": bv_t,
            "masks": masks_t,
        })
    return in_maps


def kernel(**inputs):
    global last_exec_time_ns
    import os

    mask = np.asarray(inputs["mask"])
    blocks, mask_tiles = _analyze_mask(mask)
    qk_bias = bool(np.any(np.asarray(inputs["bq"])) or np.any(np.asarray(inputs["bk"])))
    v_bias = bool(np.any(np.asarray(inputs["bv"])))
    key = (tuple(tuple(bl) for bl in blocks), len(mask_tiles), qk_bias, v_bias)
    if key not in _cache:
        _cache[key] = _build_nc(blocks, len(mask_tiles), qk_bias, v_bias)
    nc = _cache[key]

    in_maps = _prep_core_inputs(inputs, blocks, mask_tiles)
    trace = bool(os.environ.get("KERNEL_TRACE"))
    import time
    last_err = None
    for attempt in range(3):
        try:
            res = run_bass_kernel_spmd(nc, in_maps, list(range(NCORES)),
                                       trace=trace)
            break
        except Exception as e:  # transient NRT device-unrecoverable wedges
            last_err = e
            time.sleep(10.0)
    else:
        raise last_err
    last_exec_time_ns = res.exec_time_ns

    bo = np.asarray(inputs["bo"], np.float32)
    out = np.zeros((B, S, D), np.float32)
    for c in range(NCORES):
        bb = c // TP
        out[bb] += res.results[c]["out"].astype(np.float32)
    out += bo[None, None, :]
    return out
